# revision 3
# baseline (speedup 1.0000x reference)
"""Trainium2 Bass kernel for nn_EnsembleLoss (YOLO-style ensemble loss).

Full inputs: pred (16384, 256, 12) f32, target (16384, 256, 8) f32.
Output: scalar f32 loss.

Strategy: pure data parallel over the batch dim across 8 NeuronCores
(2048 rows/core). Each core streams its 40 MiB shard through SBUF once and
computes per-partition partial sums of the three elementwise loss terms
(conf / offset / dur, all folded into two accumulators) plus the quirky
cls term, which only involves the first 16384 flattened anchor rows =
global batch rows 0..31 (these live entirely in core 0's first tile).
The host sums the tiny [128, 4] per-core partials in float64.

Per-anchor math (k indexes the B*G*2 flattened anchors):
  d1 = gt_conf - pred_conf ; conf contributes (0.5 + 0.5*obj) * d1^2
  d2 = gt_off  - pred_off  ; offset contributes 5 * obj * d2^2
  d3 = sqrt(5*gt_dur) - sqrt(5*pred_dur) ; dur contributes obj * d3^2
  (obj = gt_conf, which is exactly 0.0 or 1.0)
accH accumulates 0.5*d1^2 over everything; accT accumulates
obj * (0.5*d1^2 + 5*d2^2 + d3^2).  loss_main = (accH + accT) / B.

cls term (rows 0..31, both anchors, all 256 grid cells = 16384 logit rows):
  l = obj * pred_cls (3 logits), contribution = lse(l) - l[int(obj*gt_cls)]
  summed via exp/reduce/ln on-device; host adds (sum lse - sum sel) / B.
"""

import math

import numpy as np

import concourse.bacc as bacc
import concourse.mybir as mybir
import concourse.tile as tile
from concourse import bass_utils

F32 = mybir.dt.float32
AF = mybir.ActivationFunctionType
OP = mybir.AluOpType
AX = mybir.AxisListType

B, G, NA = 16384, 256, 2
N_CORES = 8
SHARD = B // N_CORES          # 2048 batch rows per core
PRED_W = G * NA * 6           # 3072 f32 per batch row
TGT_W = G * NA * 4            # 2048 f32 per batch row
K = G * NA                    # 512 anchors per batch row
SQ05 = math.sqrt(0.5)
SQ5 = math.sqrt(5.0)


def build_program(rows: int = SHARD, n_devices: int = N_CORES, reps: int = 1):
    """One SPMD program: processes a [rows, ...] batch shard, writes
    out[128, 4] partial sums:
      col 0: sum 0.5*d1^2            (conf base, all elements)
      col 1: sum obj*inner           (conf obj + offset + dur)
      col 2: sum lse (rows 0..31)    (cls log-sum-exp part)
      col 3: sum sel (rows 0..31)    (cls selected-logit part)

    reps>1 repeats the streaming loop (overwriting the accumulators) —
    only used for loop-delta timing, not for correctness.
    """
    assert rows % 128 == 0
    T = rows // 128
    nc = bacc.Bacc(
        "TRN2", target_bir_lowering=False, debug=False, num_devices=n_devices
    )
    pred_d = nc.dram_tensor("pred", [rows, PRED_W], F32, kind="ExternalInput").ap()
    tgt_d = nc.dram_tensor("target", [rows, TGT_W], F32, kind="ExternalInput").ap()
    out_d = nc.dram_tensor("out", [128, 4], F32, kind="ExternalOutput").ap()

    with tile.TileContext(nc) as tc:
        with (
            tc.tile_pool(name="pin", bufs=4) as pin,
            tc.tile_pool(name="tin", bufs=4) as tin,
            tc.tile_pool(name="tmp", bufs=2) as tp,
            tc.tile_pool(name="pers", bufs=1) as pp,
        ):
            accH = pp.tile([128, T], F32, tag="accH")
            accT = pp.tile([128, T], F32, tag="accT")
            out_sb = pp.tile([128, 4], F32, tag="out_sb")
            l_t = pp.tile([32, K * 3], F32, tag="l_t")  # cls logits persist
            iot = pp.tile([128, 3], F32, tag="iot")

            nc.vector.memset(out_sb[:], 0.0)
            for c in range(3):
                nc.vector.memset(iot[:, c : c + 1], float(c))

            for t in [t for _ in range(reps) for t in range(T)]:
                pt = pin.tile([128, PRED_W], F32, tag="pt")
                tg = tin.tile([128, TGT_W], F32, tag="tg")
                nc.sync.dma_start(out=pt[:], in_=pred_d[t * 128 : (t + 1) * 128, :])
                nc.sync.dma_start(out=tg[:], in_=tgt_d[t * 128 : (t + 1) * 128, :])

                # anchor-major channel views: position = k*ch_stride + c
                pv = pt[:].rearrange("p (k c) -> p k c", k=K, c=6)
                tv = tg[:].rearrange("p (k c) -> p k c", k=K, c=4)
                po, pd, pc_ = pv[:, :, 0], pv[:, :, 1], pv[:, :, 2]
                tcf, to, td = tv[:, :, 0], tv[:, :, 2], tv[:, :, 3]

                d1 = tp.tile([128, K], F32, tag="d1")
                hsq1 = tp.tile([128, K], F32, tag="hsq1")
                d2 = tp.tile([128, K], F32, tag="d2")
                sq25 = tp.tile([128, K], F32, tag="sq25")
                sp = tp.tile([128, K], F32, tag="sp")
                st = tp.tile([128, K], F32, tag="st")
                d3 = tp.tile([128, K], F32, tag="d3")
                sq35 = tp.tile([128, K], F32, tag="sq35")
                s23 = tp.tile([128, K], F32, tag="s23")
                inner = tp.tile([128, K], F32, tag="inner")
                t2 = tp.tile([128, K], F32, tag="t2")

                nc.vector.tensor_tensor(out=d1[:], in0=tcf, in1=pc_, op=OP.subtract)
                nc.scalar.activation(
                    hsq1[:], d1[:], AF.Square, scale=SQ05,
                    accum_out=accH[:, t : t + 1],
                )
                nc.vector.tensor_tensor(out=d2[:], in0=to, in1=po, op=OP.subtract)
                nc.scalar.activation(sq25[:], d2[:], AF.Square, scale=SQ5)
                nc.scalar.activation(sp[:], pd, AF.Sqrt, scale=5.0)
                nc.scalar.activation(st[:], td, AF.Sqrt, scale=5.0)
                nc.vector.tensor_tensor(out=d3[:], in0=st[:], in1=sp[:], op=OP.subtract)
                nc.scalar.activation(sq35[:], d3[:], AF.Square)
                nc.vector.tensor_tensor(out=s23[:], in0=sq25[:], in1=sq35[:], op=OP.add)
                nc.vector.tensor_tensor(out=inner[:], in0=s23[:], in1=hsq1[:], op=OP.add)
                nc.vector.scalar_tensor_tensor(
                    out=t2[:], in0=inner[:], scalar=1.0, in1=tcf,
                    op0=OP.mult, op1=OP.mult,
                    accum_out=accT[:, t : t + 1],
                )

                if t == 0:
                    # cls term: global batch rows 0..31 (= first 16384
                    # flattened logit rows), partitions 0..31 of tile 0.
                    pcl = pt[0:32, :].rearrange(
                        "p (g a c) -> p g a c", g=G, a=NA, c=6
                    )[:, :, :, 3:6]
                    tvc = tg[0:32, :].rearrange(
                        "p (g a c) -> p g a c", g=G, a=NA, c=4
                    )
                    obj = tvc[:, :, :, 0]
                    gcls = tvc[:, :, :, 1]

                    cm = tp.tile([32, K], F32, tag="cm")
                    nc.vector.tensor_tensor(
                        out=cm[:].rearrange("p (g a) -> p g a", g=G, a=NA),
                        in0=obj, in1=gcls, op=OP.mult,
                    )
                    obj_b = obj.unsqueeze(3).broadcast_to([32, G, NA, 3])
                    lv = l_t[:].rearrange("p (g a c) -> p g a c", g=G, a=NA, c=3)
                    nc.vector.tensor_tensor(out=lv, in0=pcl, in1=obj_b, op=OP.mult)

                    cm_b = (
                        cm[:]
                        .rearrange("p (g a) -> p g a", g=G, a=NA)
                        .unsqueeze(3)
                        .broadcast_to([32, G, NA, 3])
                    )
                    iot_b = (
                        iot[0:32, :].unsqueeze(1).unsqueeze(1)
                        .broadcast_to([32, G, NA, 3])
                    )
                    mq = tp.tile([32, K * 3], F32, tag="mq")
                    nc.vector.tensor_tensor(
                        out=mq[:].rearrange("p (g a c) -> p g a c", g=G, a=NA, c=3),
                        in0=cm_b, in1=iot_b, op=OP.is_equal,
                    )
                    selt = tp.tile([32, K * 3], F32, tag="selt")
                    nc.vector.scalar_tensor_tensor(
                        out=selt[:], in0=mq[:], scalar=1.0, in1=l_t[:],
                        op0=OP.mult, op1=OP.mult,
                        accum_out=out_sb[0:32, 3:4],
                    )

            # final per-partition reductions
            nc.vector.tensor_reduce(
                out=out_sb[:, 0:1], in_=accH[:], axis=AX.X, op=OP.add
            )
            nc.vector.tensor_reduce(
                out=out_sb[:, 1:2], in_=accT[:], axis=AX.X, op=OP.add
            )

            # cls epilogue: lse = ln(sum_c exp(l)); one ACT table switch
            e_t = pp.tile([32, K * 3], F32, tag="e_t")
            nc.scalar.activation(e_t[:], l_t[:], AF.Exp)
            se = pp.tile([32, K], F32, tag="se")
            nc.vector.tensor_reduce(
                out=se[:],
                in_=e_t[:].rearrange("p (k c) -> p k c", k=K, c=3),
                axis=AX.X, op=OP.add,
            )
            lse = pp.tile([32, K], F32, tag="lse")
            nc.scalar.activation(
                lse[:], se[:], AF.Ln, accum_out=out_sb[0:32, 2:3]
            )

            nc.sync.dma_start(out=out_d, in_=out_sb[:])

    nc.compile()
    return nc


_PROGRAM = None


def _get_program():
    global _PROGRAM
    if _PROGRAM is None:
        _PROGRAM = build_program()
    return _PROGRAM


def host_reduce(outs: list[np.ndarray]) -> np.ndarray:
    """Combine per-core [128, 4] partials into the scalar loss."""
    total = 0.0
    for o in outs:
        o64 = o.astype(np.float64)
        total += o64[:, 0].sum() + o64[:, 1].sum()
    o0 = outs[0].astype(np.float64)
    total += o0[0:32, 2].sum() - o0[0:32, 3].sum()
    return np.array(total / B, dtype=np.float32)


def kernel(pred: np.ndarray, target: np.ndarray) -> np.ndarray:
    assert pred.shape == (B, G, 12) and target.shape == (B, G, 8)
    nc = _get_program()
    in_maps = [
        {
            "pred": np.ascontiguousarray(
                pred[i * SHARD : (i + 1) * SHARD].reshape(SHARD, PRED_W),
                dtype=np.float32,
            ),
            "target": np.ascontiguousarray(
                target[i * SHARD : (i + 1) * SHARD].reshape(SHARD, TGT_W),
                dtype=np.float32,
            ),
        }
        for i in range(N_CORES)
    ]
    res = bass_utils.run_bass_kernel_spmd(nc, in_maps, core_ids=list(range(N_CORES)))
    outs = [r["out"] for r in res.results]
    return host_reduce(outs)


# revision 13
# speedup vs baseline: 7.4119x; 7.4119x over previous
"""Trainium2 Bass kernel for nn_EnsembleLoss (YOLO-style ensemble loss).

Full inputs: pred (16384, 256, 12) f32, target (16384, 256, 8) f32.
Output: scalar f32 loss.

Strategy: pure data parallel over the batch dim across 8 NeuronCores
(2048 rows/core). Each core streams its 40 MiB shard through SBUF once and
computes per-partition partial sums of the three elementwise loss terms
(conf / offset / dur, folded into two accumulators) plus the quirky cls
term, which only involves the first 16384 flattened anchor rows = global
batch rows 0..31 (these live entirely in core 0's first tile). The host
sums the tiny [128, 4] per-core partials in float64.

Per-anchor math (k indexes the B*G*2 flattened anchors):
  d1 = gt_conf - pred_conf ; conf contributes (0.5 + 0.5*obj) * d1^2
  d2 = gt_off  - pred_off  ; offset contributes 5 * obj * d2^2
  d3 = sqrt(5*gt_dur) - sqrt(5*pred_dur) ; dur contributes obj * d3^2
  (obj = gt_conf, which is exactly 0.0 or 1.0)
accH accumulates 0.5*d1^2 over everything; accT accumulates
obj * (0.5*d1^2 + 5*d2^2 + d3^2).  loss_main = (accH + accT) / B.

cls term (rows 0..31, both anchors, all 256 grid cells = 16384 logit rows):
  l = obj * pred_cls (3 logits), contribution = lse(l) - l[int(obj*gt_cls)]
  summed via exp/reduce/ln on-device; host adds (sum lse - sum sel) / B.

Engine balance per 512-anchor slice: 6 DVE passes, 5 ACT passes
(squares/sqrts live on ACT — all in the `sqrt_and_others` table, so the
main loop needs no ACT table switches). DMA-bound end to end.
"""

import math

import numpy as np

import concourse.bacc as bacc
import concourse.mybir as mybir
import concourse.tile as tile
from concourse import bass_utils

F32 = mybir.dt.float32
AF = mybir.ActivationFunctionType
OP = mybir.AluOpType
AX = mybir.AxisListType

B, G, NA = 16384, 256, 2
N_CORES = 8
SHARD = B // N_CORES          # 2048 batch rows per core
PRED_W = G * NA * 6           # 3072 f32 per batch row
TGT_W = G * NA * 4            # 2048 f32 per batch row
K = G * NA                    # 512 anchors per batch row
SQ05 = math.sqrt(0.5)
SQ5 = math.sqrt(5.0)

# default build knobs (what kernel() ships)
R_DEFAULT = 1                 # batch rows per partition per tile
BUFS_DEFAULT = 4


def build_program(
    rows: int = SHARD,
    n_devices: int = N_CORES,
    reps: int = 1,
    compute: bool = True,
    R: int = R_DEFAULT,
    bufs: int = BUFS_DEFAULT,
    dual_ring: bool = False,
):
    """One SPMD program: processes a [rows, ...] batch shard, writes
    out[128, 4] partial sums:
      col 0: sum 0.5*d1^2            (conf base, all elements)
      col 1: sum obj*inner           (conf obj + offset + dur)
      col 2: sum lse (cls rows)      (cls log-sum-exp part)
      col 3: sum sel (cls rows)      (cls selected-logit part)

    reps>1 repeats the streaming loop (overwriting the accumulators) —
    only used for loop-delta timing, not for correctness.
    """
    assert rows % (128 * R) == 0
    assert 32 % R == 0
    T = rows // (128 * R)
    P = 32 // R                # partitions holding the cls rows (tile 0)
    F = R * K                  # anchors per partition per tile
    nc = bacc.Bacc(
        "TRN2", target_bir_lowering=False, debug=False, num_devices=n_devices
    )
    pred_d = nc.dram_tensor("pred", [rows, PRED_W], F32, kind="ExternalInput").ap()
    tgt_d = nc.dram_tensor("target", [rows, TGT_W], F32, kind="ExternalInput").ap()
    out_d = nc.dram_tensor("out", [128, 4], F32, kind="ExternalOutput").ap()

    tgt_dma = nc.scalar if dual_ring else nc.sync

    with tile.TileContext(nc) as tc:
        with (
            tc.tile_pool(name="pin", bufs=bufs) as pin,
            tc.tile_pool(name="tin", bufs=bufs) as tin,
            tc.tile_pool(name="tmp", bufs=2) as tp,
            tc.tile_pool(name="clsp", bufs=1) as cp,
            tc.tile_pool(name="pers", bufs=1) as pp,
        ):
            accH = pp.tile([128, T], F32, tag="accH")
            accT = pp.tile([128, T], F32, tag="accT")
            out_sb = pp.tile([128, 4], F32, tag="out_sb")
            l_t = pp.tile([P, R * G * NA * 3], F32, tag="l_t")
            iot = pp.tile([128, 3], F32, tag="iot")

            nc.vector.memset(out_sb[:], 0.0)
            nc.vector.memset(accH[:], 0.0)
            nc.vector.memset(accT[:], 0.0)
            nc.vector.memset(l_t[:], 0.0)
            for c in range(3):
                nc.vector.memset(iot[:, c : c + 1], float(c))

            for t in [t for _ in range(reps) for t in range(T)]:
                pt = pin.tile([128, R * PRED_W], F32, tag="pt")
                tg = tin.tile([128, R * TGT_W], F32, tag="tg")
                rows0 = t * 128 * R
                nc.sync.dma_start(
                    out=pt[:],
                    in_=pred_d[rows0 : rows0 + 128 * R, :].rearrange(
                        "(p r) m -> p (r m)", p=128, r=R
                    ),
                )
                tgt_dma.dma_start(
                    out=tg[:],
                    in_=tgt_d[rows0 : rows0 + 128 * R, :].rearrange(
                        "(p r) m -> p (r m)", p=128, r=R
                    ),
                )

                if not compute:
                    continue

                # anchor-major channel views: position = q*ch + c where
                # q = r*K + k runs over all F anchors with uniform stride
                pv = pt[:].rearrange("p (q c) -> p q c", q=F, c=6)
                tv = tg[:].rearrange("p (q c) -> p q c", q=F, c=4)
                po, pd, pc_ = pv[:, :, 0], pv[:, :, 1], pv[:, :, 2]
                tcf, to, td = tv[:, :, 0], tv[:, :, 2], tv[:, :, 3]

                def mk(tag):
                    return tp.tile([128, F], F32, tag=tag, name=tag)

                d1 = mk("d1")
                hsq1 = mk("hsq1")
                d2 = mk("d2")
                sq25 = mk("sq25")
                sp = mk("sp")
                st = mk("st")
                d3 = mk("d3")
                sq35 = mk("sq35")
                s23 = mk("s23")
                inner = mk("inner")
                t2 = mk("t2")

                nc.vector.tensor_tensor(out=d1[:], in0=tcf, in1=pc_, op=OP.subtract)
                nc.scalar.activation(
                    hsq1[:], d1[:], AF.Square, scale=SQ05,
                    accum_out=accH[:, t : t + 1],
                )
                nc.vector.tensor_tensor(out=d2[:], in0=to, in1=po, op=OP.subtract)
                nc.scalar.activation(sq25[:], d2[:], AF.Square, scale=SQ5)
                nc.scalar.activation(sp[:], pd, AF.Sqrt, scale=5.0)
                nc.scalar.activation(st[:], td, AF.Sqrt, scale=5.0)
                nc.vector.tensor_tensor(out=d3[:], in0=st[:], in1=sp[:], op=OP.subtract)
                nc.scalar.activation(sq35[:], d3[:], AF.Square)
                nc.vector.tensor_tensor(out=s23[:], in0=sq25[:], in1=sq35[:], op=OP.add)
                nc.vector.tensor_tensor(out=inner[:], in0=s23[:], in1=hsq1[:], op=OP.add)
                nc.vector.scalar_tensor_tensor(
                    out=t2[:], in0=inner[:], scalar=1.0, in1=tcf,
                    op0=OP.mult, op1=OP.mult,
                    accum_out=accT[:, t : t + 1],
                )

                if t == 0:
                    # cls term: global batch rows 0..31 (= first 16384
                    # flattened logit rows) = partitions 0..P-1 of tile 0.
                    # q = (r, g, a) flattened: uniform channel stride.
                    pcl = pt[0:P, :].rearrange("p (q c) -> p q c", q=F, c=6)[
                        :, :, 3:6
                    ]
                    tvc = tg[0:P, :].rearrange("p (q c) -> p q c", q=F, c=4)
                    obj = tvc[:, :, 0]
                    gcls = tvc[:, :, 1]

                    cm = cp.tile([P, F], F32, tag="cm")
                    nc.vector.tensor_tensor(out=cm[:], in0=obj, in1=gcls, op=OP.mult)
                    obj_b = obj.unsqueeze(2).broadcast_to([P, F, 3])
                    lv = l_t[:].rearrange("p (q c) -> p q c", q=F, c=3)
                    nc.vector.tensor_tensor(out=lv, in0=pcl, in1=obj_b, op=OP.mult)

                    cm_b = cm[:].unsqueeze(2).broadcast_to([P, F, 3])
                    iot_b = iot[0:P, :].unsqueeze(1).broadcast_to([P, F, 3])
                    mq = cp.tile([P, F * 3], F32, tag="mq")
                    nc.vector.tensor_tensor(
                        out=mq[:].rearrange("p (q c) -> p q c", q=F, c=3),
                        in0=cm_b, in1=iot_b, op=OP.is_equal,
                    )
                    selt = cp.tile([P, R * K * 3], F32, tag="selt")
                    nc.vector.scalar_tensor_tensor(
                        out=selt[:], in0=mq[:], scalar=1.0, in1=l_t[:],
                        op0=OP.mult, op1=OP.mult,
                        accum_out=out_sb[0:P, 3:4],
                    )

                    # cls epilogue inline (overlaps the DMA-bound main
                    # loop; costs one extra ACT table switch, hidden)
                    e_t = cp.tile([P, R * K * 3], F32, tag="e_t")
                    nc.scalar.activation(e_t[:], l_t[:], AF.Exp)
                    se = cp.tile([P, R * K], F32, tag="se")
                    nc.vector.tensor_reduce(
                        out=se[:],
                        in_=e_t[:].rearrange("p (q c) -> p q c", q=R * K, c=3),
                        axis=AX.X, op=OP.add,
                    )
                    lse = cp.tile([P, R * K], F32, tag="lse")
                    nc.scalar.activation(
                        lse[:], se[:], AF.Ln, accum_out=out_sb[0:P, 2:3]
                    )

            # final per-partition reductions
            nc.vector.tensor_reduce(
                out=out_sb[:, 0:1], in_=accH[:], axis=AX.X, op=OP.add
            )
            nc.vector.tensor_reduce(
                out=out_sb[:, 1:2], in_=accT[:], axis=AX.X, op=OP.add
            )

            nc.sync.dma_start(out=out_d, in_=out_sb[:])

    nc.compile()
    return nc


_PROGRAM = None


def _get_program():
    global _PROGRAM
    if _PROGRAM is None:
        _PROGRAM = build_program()
    return _PROGRAM


def host_reduce(outs: list[np.ndarray]) -> np.ndarray:
    """Combine per-core [128, 4] partials into the scalar loss.
    cls partials (cols 2, 3) are only meaningful on core 0; other
    partitions/cores hold zeros there by construction on core 0, and
    other cores' cls columns are ignored entirely."""
    total = 0.0
    for o in outs:
        o64 = o.astype(np.float64)
        total += o64[:, 0].sum() + o64[:, 1].sum()
    o0 = outs[0].astype(np.float64)
    total += o0[:, 2].sum() - o0[:, 3].sum()
    return np.array(total / B, dtype=np.float32)


def kernel(pred: np.ndarray, target: np.ndarray) -> np.ndarray:
    assert pred.shape == (B, G, 12) and target.shape == (B, G, 8)
    nc = _get_program()
    in_maps = [
        {
            "pred": np.ascontiguousarray(
                pred[i * SHARD : (i + 1) * SHARD].reshape(SHARD, PRED_W),
                dtype=np.float32,
            ),
            "target": np.ascontiguousarray(
                target[i * SHARD : (i + 1) * SHARD].reshape(SHARD, TGT_W),
                dtype=np.float32,
            ),
        }
        for i in range(N_CORES)
    ]
    res = bass_utils.run_bass_kernel_spmd(nc, in_maps, core_ids=list(range(N_CORES)))
    outs = [r["out"] for r in res.results]
    return host_reduce(outs)


# revision 19
# speedup vs baseline: 9.7628x; 1.3172x over previous
"""Trainium2 Bass kernel for nn_EnsembleLoss (YOLO-style ensemble loss).

Full inputs: pred (16384, 256, 12) f32, target (16384, 256, 8) f32.
Output: scalar f32 loss.

Strategy: pure data parallel over the batch dim across 8 NeuronCores
(2048 rows/core). Each core streams its 40 MiB shard through SBUF once and
computes per-partition partial sums of the three elementwise loss terms
(conf / offset / dur, folded into two accumulators) plus the quirky cls
term, which only involves the first 16384 flattened anchor rows = global
batch rows 0..31 (these live entirely in core 0's first tile). The host
sums the tiny [128, 4] per-core partials in float64.

Per-anchor math (k indexes the B*G*2 flattened anchors):
  d1 = gt_conf - pred_conf ; conf contributes (0.5 + 0.5*obj) * d1^2
  d2 = gt_off  - pred_off  ; offset contributes 5 * obj * d2^2
  d3 = sqrt(5*gt_dur) - sqrt(5*pred_dur) ; dur contributes obj * d3^2
  (obj = gt_conf, which is exactly 0.0 or 1.0)
accH accumulates 0.5*d1^2 over everything; accT accumulates
obj * (0.5*d1^2 + 5*d2^2 + d3^2).  loss_main = (accH + accT) / B.

cls term (rows 0..31, both anchors, all 256 grid cells = 16384 logit rows):
  l = obj * pred_cls (3 logits), contribution = lse(l) - l[int(obj*gt_cls)]
  summed via exp/reduce/ln on-device; host adds (sum lse - sum sel) / B.

Engine balance per 512-anchor slice: 6 DVE passes, 5 ACT passes
(squares/sqrts live on ACT — all in the `sqrt_and_others` table, so the
main loop needs no ACT table switches). DMA-bound end to end.
"""

import math

import numpy as np

import concourse.bacc as bacc
import concourse.mybir as mybir
import concourse.tile as tile
from concourse import bass_utils

F32 = mybir.dt.float32
AF = mybir.ActivationFunctionType
OP = mybir.AluOpType
AX = mybir.AxisListType

B, G, NA = 16384, 256, 2
N_CORES = 8
SHARD = B // N_CORES          # 2048 batch rows per core
PRED_W = G * NA * 6           # 3072 f32 per batch row
TGT_W = G * NA * 4            # 2048 f32 per batch row
K = G * NA                    # 512 anchors per batch row
SQ05 = math.sqrt(0.5)
SQ5 = math.sqrt(5.0)

# default build knobs (what kernel() ships)
R_DEFAULT = 1                 # batch rows per partition per tile
BUFS_DEFAULT = 4


def build_program(
    rows: int = SHARD,
    n_devices: int = N_CORES,
    reps: int = 1,
    compute: bool = True,
    R: int = R_DEFAULT,
    bufs: int = BUFS_DEFAULT,
    dual_ring: bool = False,
):
    """One SPMD program: processes a [rows, ...] batch shard, writes
    out[128, 4] partial sums:
      col 0: sum 0.5*d1^2            (conf base, all elements)
      col 1: sum obj*inner           (conf obj + offset + dur)
      col 2: sum lse (cls rows)      (cls log-sum-exp part)
      col 3: sum sel (cls rows)      (cls selected-logit part)

    reps>1 repeats the streaming loop (overwriting the accumulators) —
    only used for loop-delta timing, not for correctness.
    """
    assert rows % (128 * R) == 0
    assert 32 % R == 0
    T = rows // (128 * R)
    P = 32 // R                # partitions holding the cls rows (tile 0)
    F = R * K                  # anchors per partition per tile
    nc = bacc.Bacc(
        "TRN2", target_bir_lowering=False, debug=False, num_devices=n_devices
    )
    pred_d = nc.dram_tensor("pred", [rows, PRED_W], F32, kind="ExternalInput").ap()
    tgt_d = nc.dram_tensor("target", [rows, TGT_W], F32, kind="ExternalInput").ap()
    out_d = nc.dram_tensor("out", [128, 4], F32, kind="ExternalOutput").ap()

    tgt_dma = nc.scalar if dual_ring else nc.sync

    with tile.TileContext(nc) as tc:
        with (
            tc.tile_pool(name="pin", bufs=bufs) as pin,
            tc.tile_pool(name="tin", bufs=bufs) as tin,
            tc.tile_pool(name="tmp", bufs=2) as tp,
            tc.tile_pool(name="clsp", bufs=1) as cp,
            tc.tile_pool(name="pers", bufs=1) as pp,
        ):
            accH = pp.tile([128, T], F32, tag="accH")
            accT = pp.tile([128, 3 * T], F32, tag="accT")
            out_sb = pp.tile([128, 4], F32, tag="out_sb")
            l_t = pp.tile([P, R * G * NA * 3], F32, tag="l_t")
            iot = pp.tile([128, 3], F32, tag="iot")

            nc.vector.memset(out_sb[:], 0.0)
            nc.vector.memset(accH[:], 0.0)
            nc.vector.memset(accT[:], 0.0)
            nc.vector.memset(l_t[:], 0.0)
            for c in range(3):
                nc.vector.memset(iot[:, c : c + 1], float(c))

            for t in [t for _ in range(reps) for t in range(T)]:
                pt = pin.tile([128, R * PRED_W], F32, tag="pt")
                tg = tin.tile([128, R * TGT_W], F32, tag="tg")
                rows0 = t * 128 * R
                nc.sync.dma_start(
                    out=pt[:],
                    in_=pred_d[rows0 : rows0 + 128 * R, :].rearrange(
                        "(p r) m -> p (r m)", p=128, r=R
                    ),
                )
                tgt_dma.dma_start(
                    out=tg[:],
                    in_=tgt_d[rows0 : rows0 + 128 * R, :].rearrange(
                        "(p r) m -> p (r m)", p=128, r=R
                    ),
                )

                if not compute:
                    continue

                # anchor-major channel views: position = q*ch + c where
                # q = r*K + k runs over all F anchors with uniform stride
                pv = pt[:].rearrange("p (q c) -> p q c", q=F, c=6)
                tv = tg[:].rearrange("p (q c) -> p q c", q=F, c=4)
                po, pd, pc_ = pv[:, :, 0], pv[:, :, 1], pv[:, :, 2]
                tcf, to, td = tv[:, :, 0], tv[:, :, 2], tv[:, :, 3]

                def mk(tag):
                    return tp.tile([128, F], F32, tag=tag, name=tag)

                d1 = mk("d1")
                hsq1 = mk("hsq1")
                d2 = mk("d2")
                sq25 = mk("sq25")
                sp = mk("sp")
                st = mk("st")
                d3 = mk("d3")
                sq35 = mk("sq35")
                tc1 = mk("tc1")
                tc2 = mk("tc2")
                tc3 = mk("tc3")

                # three independent accumulate chains (shallow critical
                # path; the scheduler overlaps them freely):
                #   conf: d1 -> 0.5*d1^2 (+accH) -> *obj (+accT)
                #   off:  d2 -> 5*d2^2            -> *obj (+accT)
                #   dur:  sqrt,sqrt -> d3 -> d3^2 -> *obj (+accT)
                nc.vector.tensor_tensor(out=d1[:], in0=tcf, in1=pc_, op=OP.subtract)
                nc.scalar.activation(
                    hsq1[:], d1[:], AF.Square, scale=SQ05,
                    accum_out=accH[:, t : t + 1],
                )
                nc.vector.scalar_tensor_tensor(
                    out=tc1[:], in0=hsq1[:], scalar=1.0, in1=tcf,
                    op0=OP.mult, op1=OP.mult,
                    accum_out=accT[:, 3 * t : 3 * t + 1],
                )
                nc.vector.tensor_tensor(out=d2[:], in0=to, in1=po, op=OP.subtract)
                nc.scalar.activation(sq25[:], d2[:], AF.Square, scale=SQ5)
                nc.vector.scalar_tensor_tensor(
                    out=tc2[:], in0=sq25[:], scalar=1.0, in1=tcf,
                    op0=OP.mult, op1=OP.mult,
                    accum_out=accT[:, 3 * t + 1 : 3 * t + 2],
                )
                nc.scalar.activation(sp[:], pd, AF.Sqrt, scale=5.0)
                nc.scalar.activation(st[:], td, AF.Sqrt, scale=5.0)
                nc.vector.tensor_tensor(out=d3[:], in0=st[:], in1=sp[:], op=OP.subtract)
                nc.scalar.activation(sq35[:], d3[:], AF.Square)
                nc.vector.scalar_tensor_tensor(
                    out=tc3[:], in0=sq35[:], scalar=1.0, in1=tcf,
                    op0=OP.mult, op1=OP.mult,
                    accum_out=accT[:, 3 * t + 2 : 3 * t + 3],
                )

                if t == 0:
                    # cls term: global batch rows 0..31 (= first 16384
                    # flattened logit rows) = partitions 0..P-1 of tile 0.
                    # q = (r, g, a) flattened: uniform channel stride.
                    pcl = pt[0:P, :].rearrange("p (q c) -> p q c", q=F, c=6)[
                        :, :, 3:6
                    ]
                    tvc = tg[0:P, :].rearrange("p (q c) -> p q c", q=F, c=4)
                    obj = tvc[:, :, 0]
                    gcls = tvc[:, :, 1]

                    cm = cp.tile([P, F], F32, tag="cm")
                    nc.vector.tensor_tensor(out=cm[:], in0=obj, in1=gcls, op=OP.mult)
                    obj_b = obj.unsqueeze(2).broadcast_to([P, F, 3])
                    lv = l_t[:].rearrange("p (q c) -> p q c", q=F, c=3)
                    nc.vector.tensor_tensor(out=lv, in0=pcl, in1=obj_b, op=OP.mult)

                    cm_b = cm[:].unsqueeze(2).broadcast_to([P, F, 3])
                    iot_b = iot[0:P, :].unsqueeze(1).broadcast_to([P, F, 3])
                    mq = cp.tile([P, F * 3], F32, tag="mq")
                    nc.vector.tensor_tensor(
                        out=mq[:].rearrange("p (q c) -> p q c", q=F, c=3),
                        in0=cm_b, in1=iot_b, op=OP.is_equal,
                    )
                    selt = cp.tile([P, R * K * 3], F32, tag="selt")
                    nc.vector.scalar_tensor_tensor(
                        out=selt[:], in0=mq[:], scalar=1.0, in1=l_t[:],
                        op0=OP.mult, op1=OP.mult,
                        accum_out=out_sb[0:P, 3:4],
                    )

                    # cls epilogue inline (overlaps the DMA-bound main
                    # loop; costs one extra ACT table switch, hidden)
                    e_t = cp.tile([P, R * K * 3], F32, tag="e_t")
                    nc.scalar.activation(e_t[:], l_t[:], AF.Exp)
                    se = cp.tile([P, R * K], F32, tag="se")
                    nc.vector.tensor_reduce(
                        out=se[:],
                        in_=e_t[:].rearrange("p (q c) -> p q c", q=R * K, c=3),
                        axis=AX.X, op=OP.add,
                    )
                    lse = cp.tile([P, R * K], F32, tag="lse")
                    nc.scalar.activation(
                        lse[:], se[:], AF.Ln, accum_out=out_sb[0:P, 2:3]
                    )

            # final per-partition reductions
            nc.vector.tensor_reduce(
                out=out_sb[:, 0:1], in_=accH[:], axis=AX.X, op=OP.add
            )
            nc.vector.tensor_reduce(
                out=out_sb[:, 1:2], in_=accT[:], axis=AX.X, op=OP.add
            )

            nc.sync.dma_start(out=out_d, in_=out_sb[:])

    nc.compile()
    return nc


_PROGRAM = None


def _get_program():
    global _PROGRAM
    if _PROGRAM is None:
        _PROGRAM = build_program()
    return _PROGRAM


def host_reduce(outs: list[np.ndarray]) -> np.ndarray:
    """Combine per-core [128, 4] partials into the scalar loss.
    cls partials (cols 2, 3) are only meaningful on core 0; other
    partitions/cores hold zeros there by construction on core 0, and
    other cores' cls columns are ignored entirely."""
    total = 0.0
    for o in outs:
        o64 = o.astype(np.float64)
        total += o64[:, 0].sum() + o64[:, 1].sum()
    o0 = outs[0].astype(np.float64)
    total += o0[:, 2].sum() - o0[:, 3].sum()
    return np.array(total / B, dtype=np.float32)


def kernel(pred: np.ndarray, target: np.ndarray) -> np.ndarray:
    pred = np.asarray(pred, dtype=np.float32)
    target = np.asarray(target, dtype=np.float32)
    assert pred.shape == (B, G, 12) and target.shape == (B, G, 8)
    nc = _get_program()
    in_maps = [
        {
            "pred": np.ascontiguousarray(
                pred[i * SHARD : (i + 1) * SHARD].reshape(SHARD, PRED_W),
                dtype=np.float32,
            ),
            "target": np.ascontiguousarray(
                target[i * SHARD : (i + 1) * SHARD].reshape(SHARD, TGT_W),
                dtype=np.float32,
            ),
        }
        for i in range(N_CORES)
    ]
    res = bass_utils.run_bass_kernel_spmd(nc, in_maps, core_ids=list(range(N_CORES)))
    outs = [r["out"] for r in res.results]
    return host_reduce(outs)
